# revision 7
# baseline (speedup 1.0000x reference)
"""Self-attention (Content_SA) Trainium2 Bass kernel, 8-core SPMD.

Problem: B=4, C=512, H=W=64 (HW=4096) content self-attention:
  norm = instance_norm(x); F = f(norm); G = g(norm); Hf = h(x)
  energy[m,n] = F[:,m].G[:,n]; att = softmax_n(energy); out = o(Hf @ att^T) + x

Sharding: data-parallel over batch (b = core//2) x attention-row halves
(h = core%2, m_slice of 2048 rows).  Each core gets its batch's content
ROLLED by -2048*h along the spatial axis so its m_slice is always columns
[0:2048] under a single SPMD program (n-summation order is roll-invariant).
Full 1x1-conv weights are replicated to every core; no collectives.

On-core pipeline (flash-style: the 4096x2048 attention slice never leaves
the chip): instance-norm stats via bn_stats; convs G/F/HT as fp16 matmuls.
HT = Hf^T is produced directly in [n, c] layout (so the PV matmul needs no
Hf transpose) from norm16 with rstd-scaled weights + mean-correction row:
  Hf[c,n] = sum_k h_w[c,k] x[k,n] = sum_k (h_w[c,k] sd_k) norm[k,n] + hconst[c]
Energy e[m,n] tiles in [m-partition, n-free] layout -> exact row-max softmax
with ACT Exp (per-partition bias, fused accum row-sums); P normalized
(gpsimd) then PE-transposed 128x128 -> PV matmul accumulating in PSUM;
o-conv + bias + residual, DMA out.  fp16 operands / fp32 PSUM throughout.

Walrus in this container caps sync waits at 1 per instruction; Tile can
emit more (tail drain, multi-queue DMA deps), so split_excess_waits()
rewrites the module, hoisting excess waits onto preceding NoOps.
"""

import contextlib

import numpy as np

import concourse.bass as bass
import concourse.tile as tile
from concourse import mybir
from concourse.bass_utils import run_bass_kernel_spmd
from concourse.masks import make_identity

P = 128          # partitions
C = 512          # channels
HW = 4096        # spatial (64*64)
MSL = 2048       # per-core attention-row slice
NCORES = 8
EPS = 1e-5
KC = C // P      # 4 contraction chunks
NB = HW // 512   # 8 n-blocks of 512
NT = HW // P     # 32 n-chunks of 128
F16 = mybir.dt.float16
F32 = mybir.dt.float32
AX = mybir.AxisListType.X
ACT = mybir.ActivationFunctionType
ALU = mybir.AluOpType


def split_excess_waits(nc, max_waits=1):
    """Walrus here rejects >1 sync wait per instruction; hoist extras to NoOps."""
    n = 0
    for fn in nc.m.functions:
        for blk in fn.blocks:
            out = []
            for ins in blk.instructions:
                si = ins.sync_info
                if si is not None and si.on_wait and len(si.on_wait) > max_waits:
                    waits = list(si.on_wait)
                    excess, keep = waits[:-max_waits], waits[-max_waits:]
                    for i, w in enumerate(excess):
                        out.append(mybir.InstNoOp(
                            name=f"{ins.name}_ws{i}", ins=[], outs=[],
                            engine=ins.engine,
                            sync_info=mybir.SyncInfo(on_wait=[w], on_update=[])))
                        n += 1
                    ins.sync_info = mybir.SyncInfo(
                        on_wait=keep, on_update=list(si.on_update or []))
                out.append(ins)
            blk.instructions[:] = out
    return n


def build_kernel():
    nc = bass.Bass()
    x_d = nc.declare_dram_parameter("content", [C, HW], F32, isOutput=False)
    w_d = {k: nc.declare_dram_parameter(f"{k}_w", [C, C], F32, isOutput=False)
           for k in "fgho"}
    b_d = {k: nc.declare_dram_parameter(f"{k}_b", [C], F32, isOutput=False)
           for k in "fgho"}
    out_d = nc.declare_dram_parameter("out", [C, MSL], F32, isOutput=True)

    with tile.TileContext(nc) as tc:
        _emit(nc, tc, x_d, w_d, b_d, out_d)
    split_excess_waits(nc)
    return nc


def _emit(nc, tc, x_d, w_d, b_d, out_d):
    ctx = contextlib.ExitStack()
    with ctx:
        # ---------------- persistent pools ----------------
        consts = ctx.enter_context(tc.tile_pool(name="consts", bufs=1))
        stat = ctx.enter_context(tc.tile_pool(name="stat", bufs=4))
        musd = ctx.enter_context(tc.tile_pool(name="musd", bufs=1))
        wt_ho = ctx.enter_context(tc.tile_pool(name="wt_ho", bufs=1))
        gpool = ctx.enter_context(tc.tile_pool(name="gpool", bufs=1))
        fpool = ctx.enter_context(tc.tile_pool(name="fpool", bufs=1))
        htpool = ctx.enter_context(tc.tile_pool(name="htpool", bufs=1))
        rpool = ctx.enter_context(tc.tile_pool(name="rpool", bufs=1))
        atpool = ctx.enter_context(tc.tile_pool(name="atpool", bufs=8))
        fin = ctx.enter_context(tc.tile_pool(name="fin", bufs=3))
        psA = ctx.enter_context(tc.tile_pool(name="psA", bufs=5, space="PSUM"))
        psT = ctx.enter_context(tc.tile_pool(name="psT", bufs=3, space="PSUM"))

        ident = consts.tile([P, P], F16)
        make_identity(nc, ident)
        eps_t = consts.tile([P, 1], F32)
        nc.vector.memset(eps_t, EPS)
        ones1 = consts.tile([1, P], F16)
        nc.vector.memset(ones1, 1.0)

        bias_t = {}
        for k in "fgo":
            for ot in range(KC):
                t = consts.tile([P, 1], F32, tag=f"b_{k}{ot}", name=f"b_{k}{ot}")
                nc.sync.dma_start(
                    out=t,
                    in_=b_d[k].rearrange("(a b) -> a b", b=1)[ot * P:(ot + 1) * P, :])
                bias_t[(k, ot)] = t
        hb_bc = consts.tile([P, C], F32)
        nc.sync.dma_start(
            out=hb_bc, in_=bass.AP(tensor=b_d["h"], offset=0, ap=[[0, P], [1, C]]))
        hb2_bc = consts.tile([P, C], F32)   # hb + broadcast(hconst), filled later

        mu_t = [musd.tile([P, 1], F32, tag=f"mu{i}", name=f"mu{i}") for i in range(KC)]
        sd_t = [musd.tile([P, 1], F32, tag=f"sd{i}", name=f"sd{i}") for i in range(KC)]

        # h-scaled (for HT-from-norm) and o weights persist into phase B
        h_sc = [wt_ho.tile([P, C], F16, tag=f"hs{i}", name=f"h_sc{i}") for i in range(KC)]
        o_wT = [wt_ho.tile([P, C], F16, tag=f"ow{i}", name=f"o_wT{i}") for i in range(KC)]

        G16 = [gpool.tile([P, HW], F16, tag=f"G{i}", name=f"G16_{i}") for i in range(KC)]
        F16t = [fpool.tile([P, MSL], F16, tag=f"F{i}", name=f"F16_{i}") for i in range(KC)]
        HT16 = htpool.tile([P, NT, C], F16)
        resid = [rpool.tile([P, MSL], F16, tag=f"r{i}", name=f"resid{i}") for i in range(KC)]

        # ---------------- phase A: weights, norm, convs ----------------
        with tc.tile_pool(name="wpool", bufs=2) as wpool, \
             tc.tile_pool(name="wt_fgh", bufs=1) as wt_fgh, \
             tc.tile_pool(name="x32p", bufs=1) as x32p, \
             tc.tile_pool(name="n16p", bufs=1) as n16p:

            # weights: load fp32, cast fp16, PE-transpose to [k, o] chunks
            wT = {}
            for k in "fgh":
                for kcid in range(KC):
                    wT[(k, kcid)] = wt_fgh.tile(
                        [P, C], F16, tag=f"wT_{k}{kcid}", name=f"wT_{k}{kcid}")
            for kcid in range(KC):
                wT[("o", kcid)] = o_wT[kcid]
            for k in "fgho":
                for ot in range(KC):
                    w32 = wpool.tile([P, C], F32, tag="w32")
                    nc.sync.dma_start(out=w32, in_=w_d[k][ot * P:(ot + 1) * P, :])
                    w16 = wpool.tile([P, C], F16, tag="w16")
                    nc.vector.tensor_copy(w16, w32)
                    for kcid in range(KC):
                        tp = psT.tile([P, P], F16)
                        nc.tensor.transpose(tp, w16[:, kcid * P:(kcid + 1) * P], ident)
                        nc.scalar.copy(wT[(k, kcid)][:, ot * P:(ot + 1) * P], tp)

            # content: stats + norm16 (x32 streamed, never kept)
            norm16 = [n16p.tile([P, HW], F16, tag=f"n{i}", name=f"norm16_{i}")
                      for i in range(KC)]
            for ct in range(KC):
                x32 = x32p.tile([P, HW], F32, tag="x32")
                nc.sync.dma_start(out=x32, in_=x_d[ct * P:(ct + 1) * P, :])
                st = stat.tile([P, 8, 6], F32, tag="bnst")
                xv = x32.rearrange("p (s q) -> p s q", q=512)
                for s in range(8):
                    nc.vector.bn_stats(st[:, s, :], xv[:, s, :])
                mv = stat.tile([P, 2], F32, tag="mv")
                nc.vector.bn_aggr(mv, st)
                nc.gpsimd.tensor_copy(mu_t[ct], mv[:, 0:1])
                nc.scalar.activation(out=sd_t[ct], in_=mv[:, 1:2], func=ACT.Sqrt,
                                     bias=eps_t, scale=1.0)
                rstd = stat.tile([P, 1], F32, tag="rstd")
                nc.vector.reciprocal(rstd, sd_t[ct])
                nc.vector.tensor_scalar(
                    out=norm16[ct], in0=x32, scalar1=mv[:, 0:1], scalar2=rstd,
                    op0=ALU.subtract, op1=ALU.mult)
                # residual slice: x = norm*sd + mu (fp16)
                nc.vector.tensor_scalar(
                    out=resid[ct], in0=norm16[ct][:, :MSL],
                    scalar1=sd_t[ct], scalar2=mu_t[ct],
                    op0=ALU.mult, op1=ALU.add)
                # h-weights scaled by sd_k so HT can be computed from norm16
                nc.gpsimd.tensor_scalar(
                    out=h_sc[ct], in0=wT[("h", ct)], scalar1=sd_t[ct],
                    scalar2=None, op0=ALU.mult)

            # hconst[c] = sum_k mu_k h_w[c,k]; hb2_bc = hb + broadcast(hconst)
            mu16 = consts.tile([P, KC], F16)
            for kcid in range(KC):
                nc.gpsimd.tensor_copy(mu16[:, kcid:kcid + 1], mu_t[kcid])
            hc_ps = psA.tile([1, C], F32, tag="ps", name="hc_ps")
            for kcid in range(KC):
                nc.tensor.matmul(hc_ps, mu16[:, kcid:kcid + 1], wT[("h", kcid)],
                                 start=(kcid == 0), stop=(kcid == KC - 1))
            hc16 = consts.tile([1, C], F16)
            nc.vector.tensor_copy(hc16, hc_ps)
            bc_ps = psA.tile([P, C], F32, tag="ps", name="bc_ps")
            nc.tensor.matmul(bc_ps, ones1, hc16, start=True, stop=True)
            nc.vector.tensor_add(hb2_bc, hb_bc, bc_ps)

            # convs: G (full), F (m-slice)
            for ot in range(KC):
                for nb in range(NB):
                    ps = psA.tile([P, 512], F32)
                    for kcid in range(KC):
                        nc.tensor.matmul(
                            ps, wT[("g", kcid)][:, ot * P:(ot + 1) * P],
                            norm16[kcid][:, nb * 512:(nb + 1) * 512],
                            start=(kcid == 0), stop=(kcid == KC - 1))
                    nc.vector.tensor_scalar(
                        out=G16[ot][:, nb * 512:(nb + 1) * 512], in0=ps,
                        scalar1=bias_t[("g", ot)], scalar2=None, op0=ALU.add)
            for ot in range(KC):
                for mb in range(MSL // 512):
                    ps = psA.tile([P, 512], F32)
                    for kcid in range(KC):
                        nc.tensor.matmul(
                            ps, wT[("f", kcid)][:, ot * P:(ot + 1) * P],
                            norm16[kcid][:, mb * 512:(mb + 1) * 512],
                            start=(kcid == 0), stop=(kcid == KC - 1))
                    nc.vector.tensor_scalar(
                        out=F16t[ot][:, mb * 512:(mb + 1) * 512], in0=ps,
                        scalar1=bias_t[("f", ot)], scalar2=None, op0=ALU.add)

            # HT[n, c] = sum_k norm[k, n] * (h_w[c, k] sd_k)  + (hconst + h_b)[c]
            for nt in range(NT):
                ps = psA.tile([P, 512], F32)
                for kcid in range(KC):
                    nc.tensor.matmul(
                        ps, norm16[kcid][:, nt * P:(nt + 1) * P], h_sc[kcid],
                        start=(kcid == 0), stop=(kcid == KC - 1))
                nc.vector.tensor_add(HT16[:, nt, :], ps, hb2_bc)

        # ---------------- phase B: attention ----------------
        with tc.tile_pool(name="ptpool", bufs=1) as ptpool, \
             tc.tile_pool(name="epool", bufs=1) as epool, \
             tc.tile_pool(name="ppool", bufs=1) as ppool:
            for mb in range(MSL // 512):
                PT = [ptpool.tile([P, 8, 512], F16, tag=f"PT{i}", name=f"PT_{mb}_{i}")
                      for i in range(4)]
                for sub in range(4):
                    mt = mb * 4 + sub
                    e_sb = epool.tile([P, HW], F32, tag="e", name=f"e_{mt}")
                    for nb in range(NB):
                        ps = psA.tile([P, 512], F32)
                        for kcid in range(KC):
                            nc.tensor.matmul(
                                ps, F16t[kcid][:, mt * P:(mt + 1) * P],
                                G16[kcid][:, nb * 512:(nb + 1) * 512],
                                start=(kcid == 0), stop=(kcid == KC - 1))
                        nc.scalar.copy(e_sb[:, nb * 512:(nb + 1) * 512], ps)
                    negmax = stat.tile([P, 1], F32, tag="negmax")
                    nc.vector.reduce_max(negmax, e_sb, axis=AX, negate=True)
                    p16 = ppool.tile([P, HW], F16, tag="p16", name=f"p16_{mt}")
                    rowsum = stat.tile([P, 1], F32, tag="rowsum")
                    nc.scalar.activation(out=p16, in_=e_sb, func=ACT.Exp,
                                         bias=negmax, scale=1.0, accum_out=rowsum)
                    recip = stat.tile([P, 1], F32, tag="recip")
                    nc.vector.reciprocal(recip, rowsum)
                    nc.gpsimd.tensor_scalar(
                        out=p16, in0=p16, scalar1=recip, scalar2=None, op0=ALU.mult)
                    for nt in range(NT):
                        tp = psT.tile([P, P], F16)
                        nc.tensor.transpose(tp, p16[:, nt * P:(nt + 1) * P], ident)
                        nc.vector.tensor_copy(
                            PT[nt // 8][:, nt % 8, sub * P:(sub + 1) * P], tp)

                att16 = [atpool.tile([P, 512], F16, tag="att", name=f"att_{mb}_{i}")
                         for i in range(KC)]
                for ci in range(KC):
                    ps = psA.tile([P, 512], F32)
                    for nt in range(NT):
                        nc.tensor.matmul(
                            ps, HT16[:, nt, ci * P:(ci + 1) * P],
                            PT[nt // 8][:, nt % 8, :],
                            start=(nt == 0), stop=(nt == NT - 1))
                    nc.vector.tensor_copy(att16[ci], ps)

                for oi in range(KC):
                    ps = psA.tile([P, 512], F32)
                    for ci in range(KC):
                        nc.tensor.matmul(
                            ps, o_wT[ci][:, oi * P:(oi + 1) * P], att16[ci],
                            start=(ci == 0), stop=(ci == KC - 1))
                    o_sb = fin.tile([P, 512], F32, tag="osb")
                    nc.vector.tensor_scalar(
                        out=o_sb, in0=ps, scalar1=bias_t[("o", oi)],
                        scalar2=None, op0=ALU.add)
                    nc.vector.tensor_add(
                        o_sb, o_sb, resid[oi][:, mb * 512:(mb + 1) * 512])
                    nc.sync.dma_start(
                        out=out_d[oi * P:(oi + 1) * P, mb * 512:(mb + 1) * 512],
                        in_=o_sb)


_NC_CACHE = None


def _get_nc():
    global _NC_CACHE
    if _NC_CACHE is None:
        _NC_CACHE = build_kernel()
    return _NC_CACHE


def kernel(content_feat, f_w, f_b, g_w, g_b, h_w, h_b, o_w, o_b):
    content_feat = np.ascontiguousarray(np.asarray(content_feat, dtype=np.float32))
    B, Cc, Hh, Ww = content_feat.shape
    assert (B, Cc, Hh * Ww) == (4, C, HW)
    flat = content_feat.reshape(B, C, HW)

    weights = {
        "f_w": np.ascontiguousarray(np.asarray(f_w, np.float32)),
        "g_w": np.ascontiguousarray(np.asarray(g_w, np.float32)),
        "h_w": np.ascontiguousarray(np.asarray(h_w, np.float32)),
        "o_w": np.ascontiguousarray(np.asarray(o_w, np.float32)),
        "f_b": np.ascontiguousarray(np.asarray(f_b, np.float32)),
        "g_b": np.ascontiguousarray(np.asarray(g_b, np.float32)),
        "h_b": np.ascontiguousarray(np.asarray(h_b, np.float32)),
        "o_b": np.ascontiguousarray(np.asarray(o_b, np.float32)),
    }

    in_maps = []
    for core in range(NCORES):
        b, h = core // 2, core % 2
        rolled = np.ascontiguousarray(np.roll(flat[b], -MSL * h, axis=1))
        in_maps.append({"content": rolled, **weights})

    nc = _get_nc()
    res = run_bass_kernel_spmd(nc, in_maps, list(range(NCORES)))

    out = np.empty((B, C, HW), dtype=np.float32)
    for core in range(NCORES):
        b, h = core // 2, core % 2
        out[b][:, MSL * h:MSL * (h + 1)] = res.results[core]["out"]
    return out.reshape(B, C, Hh, Ww)


# revision 13
# speedup vs baseline: 1.1966x; 1.1966x over previous
"""Self-attention (Content_SA) Trainium2 Bass kernel, 8-core SPMD.

Problem: B=4, C=512, H=W=64 (HW=4096) content self-attention:
  norm = instance_norm(x); F = f(norm); G = g(norm); Hf = h(x)
  energy[m,n] = F[:,m].G[:,n]; att = softmax_n(energy); out = o(Hf @ att^T) + x

Sharding: data-parallel over batch (b = core//2) x attention-row halves
(h = core%2, m_slice of 2048 rows).  Each core gets its batch's content
ROLLED by -2048*h along the spatial axis so its m_slice is always columns
[0:2048] under a single SPMD program (n-summation order is roll-invariant).
Full 1x1-conv weights are replicated to every core; no collectives.

On-core pipeline (flash-style: the 4096x2048 attention slice never leaves
the chip): instance-norm stats via bn_stats; convs G/F/HT as fp16 matmuls.
HT = Hf^T is produced directly in [n, c] layout (so the PV matmul needs no
Hf transpose) from norm16 with rstd-scaled weights + mean-correction row:
  Hf[c,n] = sum_k h_w[c,k] x[k,n] = sum_k (h_w[c,k] sd_k) norm[k,n] + hconst[c]
Energy e[m,n] tiles in [m-partition, n-free] layout -> exact row-max softmax
with ACT Exp (per-partition bias, fused accum row-sums); P normalized
(gpsimd) then PE-transposed 128x128 -> PV matmul accumulating in PSUM;
o-conv + bias + residual, DMA out.  fp16 operands / fp32 PSUM throughout.

Walrus in this container caps sync waits at 1 per instruction; Tile can
emit more (tail drain, multi-queue DMA deps), so split_excess_waits()
rewrites the module, hoisting excess waits onto preceding NoOps.
"""

import contextlib

import numpy as np

import concourse.bass as bass
import concourse.tile as tile
from concourse import mybir
from concourse.bass_utils import run_bass_kernel_spmd
from concourse.masks import make_identity

P = 128          # partitions
C = 512          # channels
HW = 4096        # spatial (64*64)
MSL = 2048       # per-core attention-row slice
NCORES = 8
EPS = 1e-5
KC = C // P      # 4 contraction chunks
NB = HW // 512   # 8 n-blocks of 512
NT = HW // P     # 32 n-chunks of 128
F16 = mybir.dt.float16
F32 = mybir.dt.float32
AX = mybir.AxisListType.X
ACT = mybir.ActivationFunctionType
ALU = mybir.AluOpType


def split_excess_waits(nc, max_waits=1):
    """Walrus here rejects >1 sync wait per instruction; hoist extras to NoOps."""
    n = 0
    for fn in nc.m.functions:
        for blk in fn.blocks:
            out = []
            for ins in blk.instructions:
                si = ins.sync_info
                if si is not None and si.on_wait and len(si.on_wait) > max_waits:
                    waits = list(si.on_wait)
                    excess, keep = waits[:-max_waits], waits[-max_waits:]
                    for i, w in enumerate(excess):
                        out.append(mybir.InstNoOp(
                            name=f"{ins.name}_ws{i}", ins=[], outs=[],
                            engine=ins.engine,
                            sync_info=mybir.SyncInfo(on_wait=[w], on_update=[])))
                        n += 1
                    ins.sync_info = mybir.SyncInfo(
                        on_wait=keep, on_update=list(si.on_update or []))
                out.append(ins)
            blk.instructions[:] = out
    return n


def build_kernel():
    nc = bass.Bass()
    x_d = nc.declare_dram_parameter("content", [C, HW], F32, isOutput=False)
    w_d = {k: nc.declare_dram_parameter(f"{k}_w", [C, C], F32, isOutput=False)
           for k in "fgho"}
    b_d = {k: nc.declare_dram_parameter(f"{k}_b", [C], F32, isOutput=False)
           for k in "fgho"}
    out_d = nc.declare_dram_parameter("out", [C, MSL], F32, isOutput=True)

    with tile.TileContext(nc) as tc:
        _emit(nc, tc, x_d, w_d, b_d, out_d)
    split_excess_waits(nc)
    return nc


def _emit(nc, tc, x_d, w_d, b_d, out_d):
    ctx = contextlib.ExitStack()
    with ctx:
        # ---------------- persistent pools ----------------
        consts = ctx.enter_context(tc.tile_pool(name="consts", bufs=1))
        stat = ctx.enter_context(tc.tile_pool(name="stat", bufs=4))
        musd = ctx.enter_context(tc.tile_pool(name="musd", bufs=1))
        wt_ho = ctx.enter_context(tc.tile_pool(name="wt_ho", bufs=1))
        gpool = ctx.enter_context(tc.tile_pool(name="gpool", bufs=1))
        fpool = ctx.enter_context(tc.tile_pool(name="fpool", bufs=1))
        htpool = ctx.enter_context(tc.tile_pool(name="htpool", bufs=1))
        rpool = ctx.enter_context(tc.tile_pool(name="rpool", bufs=1))
        atpool = ctx.enter_context(tc.tile_pool(name="atpool", bufs=8))
        fin = ctx.enter_context(tc.tile_pool(name="fin", bufs=3))
        psA = ctx.enter_context(tc.tile_pool(name="psA", bufs=6, space="PSUM"))
        psT = ctx.enter_context(tc.tile_pool(name="psT", bufs=2, space="PSUM"))

        ident = consts.tile([P, P], F16)
        make_identity(nc, ident)
        eps_t = consts.tile([P, 1], F32)
        nc.vector.memset(eps_t, EPS)
        ones1 = consts.tile([1, P], F16)
        nc.vector.memset(ones1, 1.0)

        bias_t = {}
        for k in "fgo":
            for ot in range(KC):
                t = consts.tile([P, 1], F32, tag=f"b_{k}{ot}", name=f"b_{k}{ot}")
                nc.sync.dma_start(
                    out=t,
                    in_=b_d[k].rearrange("(a b) -> a b", b=1)[ot * P:(ot + 1) * P, :])
                bias_t[(k, ot)] = t
        hb_bc = consts.tile([P, C], F32)
        nc.sync.dma_start(
            out=hb_bc, in_=bass.AP(tensor=b_d["h"], offset=0, ap=[[0, P], [1, C]]))
        hb2_bc = consts.tile([P, C], F32)   # hb + broadcast(hconst), filled later

        mu_t = [musd.tile([P, 1], F32, tag=f"mu{i}", name=f"mu{i}") for i in range(KC)]
        sd_t = [musd.tile([P, 1], F32, tag=f"sd{i}", name=f"sd{i}") for i in range(KC)]

        # h-scaled (for HT-from-norm) and o weights persist into phase B
        h_sc = [wt_ho.tile([P, C], F16, tag=f"hs{i}", name=f"h_sc{i}") for i in range(KC)]
        o_wT = [wt_ho.tile([P, C], F16, tag=f"ow{i}", name=f"o_wT{i}") for i in range(KC)]

        G16 = [gpool.tile([P, HW], F16, tag=f"G{i}", name=f"G16_{i}") for i in range(KC)]
        F16t = [fpool.tile([P, MSL], F16, tag=f"F{i}", name=f"F16_{i}") for i in range(KC)]
        HT16 = htpool.tile([P, NT, C], F16)
        resid = [rpool.tile([P, MSL], F16, tag=f"r{i}", name=f"resid{i}") for i in range(KC)]

        # ---------------- phase A: weights, norm, convs ----------------
        with tc.tile_pool(name="wpool", bufs=2) as wpool, \
             tc.tile_pool(name="wt_fgh", bufs=1) as wt_fgh, \
             tc.tile_pool(name="x32p", bufs=1) as x32p, \
             tc.tile_pool(name="n16p", bufs=1) as n16p:

            # weights: load fp32, cast fp16, PE-transpose to [k, o] chunks
            wT = {}
            for k in "fgh":
                for kcid in range(KC):
                    wT[(k, kcid)] = wt_fgh.tile(
                        [P, C], F16, tag=f"wT_{k}{kcid}", name=f"wT_{k}{kcid}")
            for kcid in range(KC):
                wT[("o", kcid)] = o_wT[kcid]
            for k in "fgho":
                for ot in range(KC):
                    w32 = wpool.tile([P, C], F32, tag="w32")
                    nc.sync.dma_start(out=w32, in_=w_d[k][ot * P:(ot + 1) * P, :])
                    w16 = wpool.tile([P, C], F16, tag="w16")
                    nc.vector.tensor_copy(w16, w32)
                    for kcid in range(KC):
                        tp = psT.tile([P, P], F16)
                        nc.tensor.transpose(tp, w16[:, kcid * P:(kcid + 1) * P], ident)
                        nc.scalar.copy(wT[(k, kcid)][:, ot * P:(ot + 1) * P], tp)

            # content: stats + norm16 (x32 streamed, never kept)
            norm16 = [n16p.tile([P, HW], F16, tag=f"n{i}", name=f"norm16_{i}")
                      for i in range(KC)]
            for ct in range(KC):
                x32 = x32p.tile([P, HW], F32, tag="x32")
                nc.sync.dma_start(out=x32, in_=x_d[ct * P:(ct + 1) * P, :])
                st = stat.tile([P, 8, 6], F32, tag="bnst")
                xv = x32.rearrange("p (s q) -> p s q", q=512)
                for s in range(8):
                    nc.vector.bn_stats(st[:, s, :], xv[:, s, :])
                mv = stat.tile([P, 2], F32, tag="mv")
                nc.vector.bn_aggr(mv, st)
                nc.gpsimd.tensor_copy(mu_t[ct], mv[:, 0:1])
                nc.scalar.activation(out=sd_t[ct], in_=mv[:, 1:2], func=ACT.Sqrt,
                                     bias=eps_t, scale=1.0)
                rstd = stat.tile([P, 1], F32, tag="rstd")
                nc.vector.reciprocal(rstd, sd_t[ct])
                nc.vector.tensor_scalar(
                    out=norm16[ct], in0=x32, scalar1=mv[:, 0:1], scalar2=rstd,
                    op0=ALU.subtract, op1=ALU.mult)
                # residual slice: x = norm*sd + mu (fp16)
                nc.vector.tensor_scalar(
                    out=resid[ct], in0=norm16[ct][:, :MSL],
                    scalar1=sd_t[ct], scalar2=mu_t[ct],
                    op0=ALU.mult, op1=ALU.add)
                # h-weights scaled by sd_k so HT can be computed from norm16
                nc.gpsimd.tensor_scalar(
                    out=h_sc[ct], in0=wT[("h", ct)], scalar1=sd_t[ct],
                    scalar2=None, op0=ALU.mult)

            # hconst[c] = sum_k mu_k h_w[c,k]; hb2_bc = hb + broadcast(hconst)
            mu16 = consts.tile([P, KC], F16)
            for kcid in range(KC):
                nc.gpsimd.tensor_copy(mu16[:, kcid:kcid + 1], mu_t[kcid])
            hc_ps = psA.tile([1, C], F32, tag="ps", name="hc_ps")
            for kcid in range(KC):
                nc.tensor.matmul(hc_ps, mu16[:, kcid:kcid + 1], wT[("h", kcid)],
                                 start=(kcid == 0), stop=(kcid == KC - 1))
            hc16 = consts.tile([1, C], F16)
            nc.vector.tensor_copy(hc16, hc_ps)
            bc_ps = psA.tile([P, C], F32, tag="ps", name="bc_ps")
            nc.tensor.matmul(bc_ps, ones1, hc16, start=True, stop=True)
            nc.vector.tensor_add(hb2_bc, hb_bc, bc_ps)

            # convs: G (full), F (m-slice)
            for ot in range(KC):
                for nb in range(NB):
                    ps = psA.tile([P, 512], F32)
                    for kcid in range(KC):
                        nc.tensor.matmul(
                            ps, wT[("g", kcid)][:, ot * P:(ot + 1) * P],
                            norm16[kcid][:, nb * 512:(nb + 1) * 512],
                            start=(kcid == 0), stop=(kcid == KC - 1))
                    nc.vector.tensor_scalar(
                        out=G16[ot][:, nb * 512:(nb + 1) * 512], in0=ps,
                        scalar1=bias_t[("g", ot)], scalar2=None, op0=ALU.add)
            for ot in range(KC):
                for mb in range(MSL // 512):
                    ps = psA.tile([P, 512], F32)
                    for kcid in range(KC):
                        nc.tensor.matmul(
                            ps, wT[("f", kcid)][:, ot * P:(ot + 1) * P],
                            norm16[kcid][:, mb * 512:(mb + 1) * 512],
                            start=(kcid == 0), stop=(kcid == KC - 1))
                    nc.vector.tensor_scalar(
                        out=F16t[ot][:, mb * 512:(mb + 1) * 512], in0=ps,
                        scalar1=bias_t[("f", ot)], scalar2=None, op0=ALU.add)

            # HT[n, c] = sum_k norm[k, n] * (h_w[c, k] sd_k)  + (hconst + h_b)[c]
            for nt in range(NT):
                ps = psA.tile([P, 512], F32)
                for kcid in range(KC):
                    nc.tensor.matmul(
                        ps, norm16[kcid][:, nt * P:(nt + 1) * P], h_sc[kcid],
                        start=(kcid == 0), stop=(kcid == KC - 1))
                nc.vector.tensor_add(HT16[:, nt, :], ps, hb2_bc)

        # ---------------- phase B: attention ----------------
        MBS = 512                      # m-block (PV/o-conv tile width)
        with tc.tile_pool(name="ptpool", bufs=1) as ptpool, \
             tc.tile_pool(name="epool", bufs=2) as epool, \
             tc.tile_pool(name="ppool", bufs=2) as ppool:
            for mb in range(MSL // MBS):
                PT = [ptpool.tile([P, 8, MBS], F16, tag=f"PT{i}", name=f"PT_{mb}_{i}")
                      for i in range(4)]
                for sub in range(MBS // P):
                    mt = mb * (MBS // P) + sub
                    e_sb = epool.tile([P, HW], F32, tag="e", name=f"e_{mt}")
                    for nb in range(NB):
                        ps = psA.tile([P, 512], F32)
                        for kcid in range(KC):
                            nc.tensor.matmul(
                                ps, F16t[kcid][:, mt * P:(mt + 1) * P],
                                G16[kcid][:, nb * 512:(nb + 1) * 512],
                                start=(kcid == 0), stop=(kcid == KC - 1))
                        nc.scalar.copy(e_sb[:, nb * 512:(nb + 1) * 512], ps)
                    negmax = stat.tile([P, 1], F32, tag="negmax")
                    nc.vector.reduce_max(negmax, e_sb, axis=AX, negate=True)
                    p16 = ppool.tile([P, HW], F16, tag="p16", name=f"p16_{mt}")
                    rowsum = stat.tile([P, 1], F32, tag="rowsum")
                    nc.scalar.activation(out=p16, in_=e_sb, func=ACT.Exp,
                                         bias=negmax, scale=1.0, accum_out=rowsum)
                    recip = stat.tile([P, 1], F32, tag="recip")
                    nc.vector.reciprocal(recip, rowsum)
                    nc.gpsimd.tensor_scalar(
                        out=p16, in0=p16, scalar1=recip, scalar2=None, op0=ALU.mult)
                    # 8 transposes per PSUM bank, then one batched copy out
                    for q in range(4):
                        tp = psT.tile([P, 8, P], F16)
                        for j in range(8):
                            nt = q * 8 + j
                            nc.tensor.transpose(
                                tp[:, j, :], p16[:, nt * P:(nt + 1) * P], ident)
                        nc.vector.tensor_copy(
                            PT[q][:, :, sub * P:(sub + 1) * P], tp)

                att16 = [atpool.tile([P, MBS], F16, tag="att", name=f"att_{mb}_{i}")
                         for i in range(KC)]
                ops = [psA.tile([P, MBS], F32, tag="ps", name=f"ops_{mb}_{i}")
                       for i in range(KC)]
                for q in range(4):
                    for ci in range(KC):
                        for j in range(8):
                            nc.tensor.matmul(
                                ops[ci], HT16[:, q * 8 + j, ci * P:(ci + 1) * P],
                                PT[q][:, j, :],
                                start=(q == 0 and j == 0), stop=(q == 3 and j == 7))
                for ci in range(KC):
                    nc.vector.tensor_copy(att16[ci], ops[ci])

                for oi in range(KC):
                    ps = psA.tile([P, MBS], F32, tag="ps", name=f"fps_{mb}_{oi}")
                    for ci in range(KC):
                        nc.tensor.matmul(
                            ps, o_wT[ci][:, oi * P:(oi + 1) * P], att16[ci],
                            start=(ci == 0), stop=(ci == KC - 1))
                    o_sb = fin.tile([P, MBS], F32, tag="osb")
                    nc.vector.tensor_scalar(
                        out=o_sb, in0=ps, scalar1=bias_t[("o", oi)],
                        scalar2=None, op0=ALU.add)
                    nc.vector.tensor_add(
                        o_sb, o_sb, resid[oi][:, mb * MBS:(mb + 1) * MBS])
                    nc.sync.dma_start(
                        out=out_d[oi * P:(oi + 1) * P, mb * MBS:(mb + 1) * MBS],
                        in_=o_sb)


_NC_CACHE = None


def _get_nc():
    global _NC_CACHE
    if _NC_CACHE is None:
        _NC_CACHE = build_kernel()
    return _NC_CACHE


def kernel(content_feat, f_w, f_b, g_w, g_b, h_w, h_b, o_w, o_b):
    content_feat = np.ascontiguousarray(np.asarray(content_feat, dtype=np.float32))
    B, Cc, Hh, Ww = content_feat.shape
    assert (B, Cc, Hh * Ww) == (4, C, HW)
    flat = content_feat.reshape(B, C, HW)

    weights = {
        "f_w": np.ascontiguousarray(np.asarray(f_w, np.float32)),
        "g_w": np.ascontiguousarray(np.asarray(g_w, np.float32)),
        "h_w": np.ascontiguousarray(np.asarray(h_w, np.float32)),
        "o_w": np.ascontiguousarray(np.asarray(o_w, np.float32)),
        "f_b": np.ascontiguousarray(np.asarray(f_b, np.float32)),
        "g_b": np.ascontiguousarray(np.asarray(g_b, np.float32)),
        "h_b": np.ascontiguousarray(np.asarray(h_b, np.float32)),
        "o_b": np.ascontiguousarray(np.asarray(o_b, np.float32)),
    }

    in_maps = []
    for core in range(NCORES):
        b, h = core // 2, core % 2
        rolled = np.ascontiguousarray(np.roll(flat[b], -MSL * h, axis=1))
        in_maps.append({"content": rolled, **weights})

    nc = _get_nc()
    res = run_bass_kernel_spmd(nc, in_maps, list(range(NCORES)))

    out = np.empty((B, C, HW), dtype=np.float32)
    for core in range(NCORES):
        b, h = core // 2, core % 2
        out[b][:, MSL * h:MSL * (h + 1)] = res.results[core]["out"]
    return out.reshape(B, C, Hh, Ww)
